# revision 5
# baseline (speedup 1.0000x reference)
"""MAB qkv attention kernel for Trainium2 — v2 (fp16 + fp8-DoubleRow logits).

Math (per batch b):
  Q = query @ Wq.T + bq ; K = key @ Wk.T + bk
  S = (Q @ K.T) * (T/sqrt(512)) ; A = softmax(S, -1)
  out = (A @ value) @ Wo.T + bo            # raw value, V-projection unused

Implementation notes:
  - G-fusion: S = query @ G @ key.T + (Wk.T @ bq) . key, G = Wq.T @ Wk.
    bk terms cancel in the softmax.  G is scaled 2^8 on device (G'' = 256 G)
    so the fp8 splits below stay in e4m3's normal range; the 2^-8 is folded
    into the softmax temperature scale.
  - Logit chain runs as 2 passes: hi fp16 matmuls + one fp8 DoubleRow matmul
    per contraction chunk carrying both cross terms (hi*lo + lo*hi) with
    balanced per-pair scales (net scale 1), all accumulating into one PSUM
    group.  Element error ~2^-15 -> logit error ~6e-3 (tolerance 2e-2).
  - query/key arrive pre-transposed and pre-split from the host (fp16 hi +
    two e4m3 streams); value arrives bf16.  Host work is marshalling only.
  - Softmax: per 512-block DVE max on PSUM, ACT exp PSUM->SBUF bf16 with
    accumulated row sums; no f32 logit eviction at all.
  - A@value and @Wo.T run in bf16 (P is near-one-hot, bf16 is plenty).
  - PSUM: 6-bank ring (S blocks / P^T transposes / M4 out) + 2-bank ring
    shared by Qg and the O^T accumulator (M3 runs dt-outer, jt-inner).
  - Qg projection is software-pipelined one group ahead so the PE has work
    during softmax tails; Wq/Wk load in 128-row chunks to shorten startup.
"""
import os
import sys

sys.path.insert(0, "/root/.axon_site/_ro/trn_rl_repo")
sys.path.insert(1, "/opt/trn_rl_repo")
import numpy as np
import ml_dtypes

B, NQ, NK, D = 16, 2048, 2048, 512
NCORES = 8
BLOC = B // NCORES
P = 128
CO = D // P          # 4 contraction chunks
GW = 512             # i-group width
NG = NQ // GW        # 4 groups
JT = NK // P         # 16 key tiles
JB = NK // 512       # 4 key blocks
ISCALE = 1.0 / float(np.sqrt(np.float32(D)))
E4 = ml_dtypes.float8_e4m3
BF = ml_dtypes.bfloat16

_CACHE = {}


def _build():
    import concourse.mybir as mybir
    import concourse.tile as tile
    from concourse import bacc
    from concourse.masks import make_identity

    f32 = mybir.dt.float32
    fp16 = mybir.dt.float16
    bf16 = mybir.dt.bfloat16
    f8 = mybir.dt.float8e4
    AF = mybir.ActivationFunctionType
    OP = mybir.AluOpType
    DR = mybir.MatmulPerfMode.DoubleRow

    nc = bacc.Bacc(None, target_bir_lowering=False)
    qh_d = nc.dram_tensor("qh", [BLOC, D, NQ], fp16, kind="ExternalInput")
    qp_d = nc.dram_tensor("qp", [BLOC, 2, D, NQ], f8, kind="ExternalInput")
    kh_d = nc.dram_tensor("kh", [BLOC, D, NK], fp16, kind="ExternalInput")
    kp_d = nc.dram_tensor("kp", [BLOC, 2, D, NK], f8, kind="ExternalInput")
    v_d = nc.dram_tensor("vb", [BLOC, NK, D], bf16, kind="ExternalInput")
    wq_d = nc.dram_tensor("Wq", [D, D], f32, kind="ExternalInput")
    wk_d = nc.dram_tensor("Wk", [D, D], f32, kind="ExternalInput")
    wo_d = nc.dram_tensor("Wo", [D, D], f32, kind="ExternalInput")
    bq_d = nc.dram_tensor("bq", [D], f32, kind="ExternalInput")
    bo_d = nc.dram_tensor("bo", [D], f32, kind="ExternalInput")
    t_d = nc.dram_tensor("T", [1], f32, kind="ExternalInput")
    o_d = nc.dram_tensor("out", [BLOC, NQ, D], f32, kind="ExternalOutput")

    with tile.TileContext(nc) as tc:
        with (
            tc.tile_pool(name="const", bufs=1) as const,
            tc.tile_pool(name="wstage", bufs=2) as wstage,
            tc.tile_pool(name="inb2", bufs=2) as inb2,
            tc.tile_pool(name="inb1", bufs=1) as inb1,
            tc.tile_pool(name="grpin", bufs=2) as grpin,
            tc.tile_pool(name="grpqg", bufs=2) as grpqg,
            tc.tile_pool(name="grp1", bufs=1) as grp1,
            tc.tile_pool(name="pstr", bufs=4) as pstr,
            tc.tile_pool(name="outp", bufs=2) as outp,
            tc.tile_pool(name="small", bufs=4) as small,
            tc.tile_pool(name="tmp", bufs=2) as tmpp,
            tc.tile_pool(name="psS", bufs=6, space="PSUM") as psS,
            tc.tile_pool(name="psO", bufs=2, space="PSUM") as psO,
        ):
            # ---------------- constants / prologue ----------------
            id32 = const.tile([P, P], f32)
            make_identity(nc, id32)
            idbf = const.tile([P, P], bf16)
            nc.vector.tensor_copy(idbf[:], id32[:])
            ones1 = const.tile([1, P], f32)
            nc.vector.memset(ones1[:], 1.0)

            wq_sb = wstage.tile([P, CO, D], f32, tag="w")
            wk_sb = wstage.tile([P, CO, D], f32, tag="w")
            for dd in range(CO):
                nc.sync.dma_start(
                    wq_sb[:, dd, :], wq_d[dd * P:(dd + 1) * P, :]
                    .rearrange("p c -> p c"))
                nc.sync.dma_start(
                    wk_sb[:, dd, :], wk_d[dd * P:(dd + 1) * P, :]
                    .rearrange("p c -> p c"))
            bq_sb = const.tile([P, CO], f32)
            nc.sync.dma_start(bq_sb[:], bq_d.rearrange("(o p) -> p o", p=P))
            bo_row = const.tile([1, D], f32)
            nc.sync.dma_start(bo_row[:], bo_d.rearrange("(a e) -> a e", a=1))
            t_row = const.tile([1, 1], f32)
            nc.sync.dma_start(t_row[:], t_d.rearrange("(a e) -> a e", a=1))

            # Split Wq/Wk (x2^6) into fp16 hi + balanced fp8 pairs so the
            # G matmuls run 2-pass fp16+fp8-DR instead of 4-cyc/row f32.
            # G_ps accumulates 2^12 * G.
            wqh = wstage.tile([P, CO, D], fp16, tag="wh")
            wkh = wstage.tile([P, CO, D], fp16, tag="wh")
            wqp8 = wstage.tile([P, 2, CO, D], f8, tag="wp")
            wkp8 = wstage.tile([P, 2, CO, D], f8, tag="wp")
            # hi extractions first so the G fp16 pass is unblocked early;
            # fp8 lo-encodes go to DVE to keep the ACT queue short.
            for dd in range(CO):
                nc.scalar.activation(
                    wqh[:, dd, :], wq_sb[:, dd, :], AF.Copy, scale=64.0)
                nc.scalar.activation(
                    wkh[:, dd, :], wk_sb[:, dd, :], AF.Copy, scale=64.0)
            for dd in range(CO):
                # lhsT (wq) pairs [h8m; l8p]; rhs (wk) pairs [l8p; h8m] so
                # the DR mm yields the two cross terms at net 2^12 scale.
                for w_sb, wh, wp8, hx in (
                        (wq_sb, wqh, wqp8, 0), (wk_sb, wkh, wkp8, 1)):
                    nc.scalar.activation(
                        wp8[:, hx, dd, :], w_sb[:, dd, :], AF.Copy)
                    w_lo = outp.tile([P, D], f32, tag="y", name="w_lo")
                    nc.vector.scalar_tensor_tensor(
                        w_lo[:], w_sb[:, dd, :], 64.0, wh[:, dd, :],
                        op0=OP.mult, op1=OP.subtract)
                    nc.vector.tensor_scalar_mul(
                        wp8[:, 1 - hx, dd, :], w_lo[:], 64.0)

            # G'' = 2^8 * Wq.T @ Wk, split to fp16 hi + balanced fp8 pairs.
            # DR pairing: [wq_h8m; wq_l8p] x [wk_l8p; wk_h8m] (net 2^12).
            g_hi = const.tile([P, CO, D], fp16)
            g_p8 = const.tile([P, 2, CO, D], f8)
            for ct in range(CO):
                g_ps = psS.tile([P, 512], f32, tag="s")
                for dd in range(CO):
                    nc.tensor.matmul(
                        g_ps[:], wqh[:, dd, ct * P:(ct + 1) * P], wkh[:, dd, :],
                        start=(dd == 0), stop=False)
                for dd in range(CO):
                    nc.tensor.matmul(
                        g_ps[:], wqp8[:, :, dd, ct * P:(ct + 1) * P],
                        wkp8[:, :, dd, :], start=False,
                        stop=(dd == CO - 1), perf_mode=DR)
                nc.scalar.activation(
                    g_hi[:, ct, :], g_ps[:], AF.Copy, scale=2.0 ** -4)
                nc.scalar.activation(
                    g_p8[:, 0, ct, :], g_hi[:, ct, :], AF.Copy, scale=2.0 ** -8)
                g_lo = outp.tile([P, D], f32, tag="y", name="g_lo")
                nc.vector.scalar_tensor_tensor(
                    g_lo[:], g_ps[:], 2.0 ** -4, g_hi[:, ct, :],
                    op0=OP.mult, op1=OP.subtract)
                nc.vector.tensor_scalar_mul(g_p8[:, 1, ct, :], g_lo[:], 16.0)

            # u'' = 2^8 * Wk.T @ bq ; u_m10 = u'' * 2^-10
            u_sb = const.tile([P, CO], f32)
            u_m10 = const.tile([P, CO], f32)
            for ct in range(CO):
                u_ps = psS.tile([P, 512], f32, tag="s")
                for dd in range(CO):
                    nc.tensor.matmul(
                        u_ps[:, 0:1], wk_sb[:, dd, ct * P:(ct + 1) * P],
                        bq_sb[:, dd:dd + 1],
                        start=(dd == 0), stop=(dd == CO - 1))
                nc.vector.tensor_scalar_mul(u_sb[:, ct:ct + 1], u_ps[:, 0:1], 256.0)
                nc.vector.tensor_scalar_mul(u_m10[:, ct:ct + 1], u_ps[:, 0:1], 0.25)

            # WoT in bf16 via PE transpose of Wo
            wo_sb = wstage.tile([P, CO, D], f32, tag="w")
            nc.sync.dma_start(wo_sb[:], wo_d.rearrange("(o p) c -> p o c", p=P))
            wot = const.tile([P, CO, D], bf16)
            for dt in range(CO):
                t_ps = psS.tile([P, 512], f32, tag="s")
                for eo in range(CO):
                    nc.tensor.transpose(
                        t_ps[:, eo * P:(eo + 1) * P],
                        wo_sb[:, eo, dt * P:(dt + 1) * P], id32)
                nc.scalar.activation(wot[:, dt, :], t_ps[:], AF.Copy)

            # bo broadcast to [128, D]; scl = T*ISCALE*2^-8
            bo_bc = const.tile([P, D], f32)
            b_ps = psS.tile([P, 512], f32, tag="s")
            nc.tensor.matmul(b_ps[:], ones1[:], bo_row[:], start=True, stop=True)
            nc.vector.tensor_copy(bo_bc[:], b_ps[:])
            t_ps2 = psS.tile([P, 512], f32, tag="s")
            nc.tensor.matmul(t_ps2[:, 0:1], ones1[:], t_row[:], start=True, stop=True)
            scl = const.tile([P, 1], f32)
            nscl = const.tile([P, 1], f32)
            nc.vector.tensor_scalar_mul(scl[:], t_ps2[:, 0:1], ISCALE / 256.0)
            nc.vector.tensor_scalar_mul(nscl[:], t_ps2[:, 0:1], -ISCALE / 256.0)

            # ---------------- per batch ----------------
            def load_group(b, ig):
                qth = grpin.tile([P, CO, GW], fp16, tag="qh", name="qth")
                nc.sync.dma_start(
                    qth[:],
                    qh_d[b, :, ig * GW:(ig + 1) * GW]
                    .rearrange("(o p) i -> p o i", p=P))
                qtp = grpin.tile([P, 2, CO, GW], f8, tag="qp", name="qtp")
                nc.sync.dma_start(
                    qtp[:],
                    qp_d[b, :, :, ig * GW:(ig + 1) * GW]
                    .rearrange("two (o p) i -> p two o i", p=P))
                return qth, qtp

            def m1(qth, qtp):
                # QgT'' = G''.T-chunks @ queryT + u''
                qg_hi = grpqg.tile([P, CO, GW], fp16, tag="qghi", name="qg_hi")
                qg_p8 = grpqg.tile([P, 2, CO, GW], f8, tag="qgp8", name="qg_p8")
                for ct in range(CO):
                    qg_ps = psO.tile([P, 512], f32, tag="o", name="qg_ps")
                    for dd in range(CO):
                        nc.tensor.matmul(
                            qg_ps[:], g_hi[:, dd, ct * P:(ct + 1) * P],
                            qth[:, dd, :], start=(dd == 0), stop=False)
                    for dd in range(CO):
                        nc.tensor.matmul(
                            qg_ps[:], g_p8[:, :, dd, ct * P:(ct + 1) * P],
                            qtp[:, :, dd, :], start=False,
                            stop=(dd == CO - 1), perf_mode=DR)
                    nc.scalar.activation(
                        qg_hi[:, ct, :], qg_ps[:], AF.Identity,
                        bias=u_sb[:, ct:ct + 1])
                    nc.scalar.activation(
                        qg_p8[:, 0, ct, :], qg_hi[:, ct, :], AF.Copy,
                        scale=2.0 ** -10)
                    qg_lo = tmpp.tile([P, GW], f32, tag="qglo", name="qg_lo")
                    nc.vector.scalar_tensor_tensor(
                        qg_lo[:], qg_ps[:], u_sb[:, ct:ct + 1],
                        qg_hi[:, ct, :], op0=OP.add, op1=OP.subtract)
                    nc.vector.tensor_scalar_mul(
                        qg_p8[:, 1, ct, :], qg_lo[:], 4.0)
                return qg_hi, qg_p8

            gq = load_group(0, 0)
            qg_cur = None
            for b in range(BLOC):
                kth = inb2.tile([P, CO, NK], fp16, tag="kh")
                nc.sync.dma_start(kth[:], kh_d[b].rearrange("(o p) j -> p o j", p=P))
                ktp = inb1.tile([P, 2, CO, NK], f8, tag="kp")
                nc.sync.dma_start(
                    ktp[:], kp_d[b].rearrange("two (o p) j -> p two o j", p=P))
                v_sb = inb2.tile([P, JT, D], bf16, tag="v")
                nc.sync.dma_start(v_sb[:], v_d[b].rearrange("(t p) d -> p t d", p=P))

                rinv = small.tile([P, JT], f32, tag="rinv")

                if b == 0:
                    qg_cur = m1(*gq)
                    gq = load_group(0, 1)

                for ig in range(NG):
                    qg_hi, qg_p8 = qg_cur

                    # -- M2 + softmax per 128-row strip --
                    pstrips = []
                    for s in range(4):
                        strip = ig * 4 + s
                        p_bf = pstr.tile([P, NK], bf16, tag="p")
                        pstrips.append(p_bf)
                        bmx = small.tile([P, 4], f32, tag="bmx")
                        ss = small.tile([P, JB], f32, tag="ss")
                        s_blocks = []
                        for jb in range(JB):
                            s_ps = psS.tile([P, 512], f32, tag="s")
                            s_blocks.append(s_ps)
                            for ct in range(CO):
                                nc.tensor.matmul(
                                    s_ps[:],
                                    qg_hi[:, ct, s * P:(s + 1) * P],
                                    kth[:, ct, jb * 512:(jb + 1) * 512],
                                    start=(ct == 0), stop=False)
                            for ct in range(CO):
                                nc.tensor.matmul(
                                    s_ps[:],
                                    qg_p8[:, :, ct, s * P:(s + 1) * P],
                                    ktp[:, :, ct, jb * 512:(jb + 1) * 512],
                                    start=False, stop=(ct == CO - 1),
                                    perf_mode=DR)
                            nc.vector.reduce_max(
                                bmx[:, jb:jb + 1], s_ps[:],
                                axis=mybir.AxisListType.X)
                        mx = small.tile([P, 1], f32, tag="mx")
                        nc.vector.reduce_max(
                            mx[:, 0:1], bmx[:], axis=mybir.AxisListType.X)
                        ebias = small.tile([P, 1], f32, tag="eb")
                        nc.vector.tensor_mul(ebias[:], mx[:, 0:1], nscl[:])
                        for jb in range(JB):
                            nc.scalar.activation(
                                p_bf[:, jb * 512:(jb + 1) * 512],
                                s_blocks[jb][:],
                                AF.Exp, bias=ebias[:, 0:1], scale=scl[:, 0:1],
                                accum_out=ss[:, jb:jb + 1])
                        rt = small.tile([P, 1], f32, tag="rt")
                        nc.vector.tensor_add(rt[:], ss[:, 0:1], ss[:, 1:2])
                        nc.vector.tensor_add(rt[:], rt[:], ss[:, 2:3])
                        nc.vector.tensor_add(rt[:], rt[:], ss[:, 3:4])
                        nc.vector.reciprocal(rinv[:, strip:strip + 1], rt[:])

                    # -- hoisted M1 for the next group (fills softmax tails) --
                    nb, nig = (b, ig + 1) if ig + 1 < NG else (b + 1, 0)
                    if nb < BLOC:
                        qg_cur = m1(*gq)
                        nnb, nnig = (nb, nig + 1) if nig + 1 < NG else (nb + 1, 0)
                        if nnb < BLOC:
                            gq = load_group(nnb, nnig)

                    # -- M3: O''^T accum over j in two dt-halves --
                    pt_all = grp1.tile([P, JT, 512], bf16, tag="pt")
                    for jt in range(JT):
                        t_ps = psS.tile([P, 512], bf16, tag="s")
                        for s in range(4):
                            nc.tensor.transpose(
                                t_ps[:, s * P:(s + 1) * P],
                                pstrips[s][:, jt * P:(jt + 1) * P], idbf)
                        nc.vector.tensor_copy(pt_all[:, jt, :], t_ps[:])
                    ot = grp1.tile([P, CO, GW], bf16, tag="ot")
                    for dt in range(CO):
                        o_ps = psO.tile([P, 512], f32, tag="o")
                        for jt in range(JT):
                            nc.tensor.matmul(
                                o_ps[:],
                                v_sb[:, jt, dt * P:(dt + 1) * P],
                                pt_all[:, jt, :],
                                start=(jt == 0), stop=(jt == JT - 1))
                        nc.scalar.activation(ot[:, dt, :], o_ps[:], AF.Copy)

                    # -- M4: out = rinv * (O''^T.T @ WoT) + bo --
                    for s in range(4):
                        strip = ig * 4 + s
                        y_ps = psS.tile([P, 512], f32, tag="s")
                        for dt in range(CO):
                            nc.tensor.matmul(
                                y_ps[:], ot[:, dt, s * P:(s + 1) * P],
                                wot[:, dt, :],
                                start=(dt == 0), stop=(dt == CO - 1))
                        y_sb = outp.tile([P, D], f32, tag="y")
                        nc.scalar.activation(
                            y_sb[:], y_ps[:], AF.Copy,
                            scale=rinv[:, strip:strip + 1])
                        nc.vector.tensor_add(y_sb[:], y_sb[:], bo_bc[:])
                        nc.sync.dma_start(
                            o_d[b, strip * P:(strip + 1) * P, :], y_sb[:])

    nc.compile()
    return nc


def _get_nc():
    if "nc" not in _CACHE:
        _CACHE["nc"] = _build()
    return _CACHE["nc"]


def _prep_host(inputs):
    """Transpose + precision-split marshalling (no FLOPs beyond rounding)."""
    q = np.asarray(inputs["query"], dtype=np.float32)
    k = np.asarray(inputs["key"], dtype=np.float32)
    v = np.asarray(inputs["value"], dtype=np.float32)

    qT = np.ascontiguousarray(q.transpose(0, 2, 1))          # [B, D, NQ]
    kT = np.ascontiguousarray(k.transpose(0, 2, 1))          # [B, D, NK]
    qh = qT.astype(np.float16)
    kh = kT.astype(np.float16)
    qp = np.empty((B, 2, D, NQ), dtype=E4)
    qp[:, 0] = ((qT - qh.astype(np.float32)) * np.float32(2.0 ** 8)).astype(E4)
    qp[:, 1] = (qT * np.float32(2.0 ** -4)).astype(E4)
    kp = np.empty((B, 2, D, NK), dtype=E4)
    kp[:, 0] = ((kT - kh.astype(np.float32)) * np.float32(2.0 ** 10)).astype(E4)
    kp[:, 1] = (kT * np.float32(2.0 ** -2)).astype(E4)
    vb = v.astype(BF)
    return qh, qp, kh, kp, vb


def kernel(**inputs):
    from concourse.bass_utils import run_bass_kernel_spmd

    nc = _get_nc()
    qh, qp, kh, kp, vb = _prep_host(inputs)
    f = lambda x: np.ascontiguousarray(np.asarray(x, dtype=np.float32))
    in_maps = []
    for c in range(NCORES):
        sl = slice(c * BLOC, (c + 1) * BLOC)
        in_maps.append({
            "qh": qh[sl], "qp": qp[sl],
            "kh": kh[sl], "kp": kp[sl],
            "vb": vb[sl],
            "Wq": f(inputs["Wq"]),
            "Wk": f(inputs["Wk"]),
            "Wo": f(inputs["Wo"]),
            "bq": f(inputs["bq"]),
            "bo": f(inputs["bo"]),
            "T": f(inputs["T"]),
        })
    res = run_bass_kernel_spmd(
        nc, in_maps, list(range(NCORES)),
        trace=bool(int(os.environ.get("KERNEL_TRACE", "0"))))
    _CACHE["last_results"] = res
    out = np.concatenate([r["out"] for r in res.results], axis=0)
    return out.astype(np.float32)


if __name__ == "__main__":
    nc = _get_nc()
    print("compiled ok")
    from profile_tl import profile
    profile(nc, "kernel_v2")
